# revision 4
# baseline (speedup 1.0000x reference)
"""Block-causal attention TRN2 kernel (8-core SPMD, head-sharded).

Problem: y = (softmax(mask(Q K^T / sqrt(d))) V) W_out + b_out where
Q,K,V = x W_qkv + b_qkv, x [2, 2048, 1024], 16 heads of d=64, block-causal
mask with chunk 128.

Sharding: core c handles batch b = c//4 and head group g = c%4 (4 heads).
Each core computes its heads' QKV projection (W_qkv column slice), the
block-causal attention, and a partial out-projection against its W_out row
slice. The host sums the 4 partial outputs per batch and adds b_out.

On-device layout is "transposed": Q^T/K^T [d, L] tiles feed scores^T
matmuls (2 heads packed in the 128-partition contraction dim), exp runs on
the scalar engine with the 1/sqrt(d) scale folded in, attn@V accumulates
with an extra ones-column of V producing the softmax denominators, and the
normalized o^T directly feeds the out-projection as the stationary operand.
All matmul operands are float32r (~1.9e-4 matmul rel err at bf16 speed).
"""

import sys

for _p in ("/opt/trn_rl_repo", "/root/.axon_site/_ro/trn_rl_repo"):
    if _p not in sys.path:
        sys.path.append(_p)

import numpy as np

import concourse.bass as bass
import concourse.mybir as mybir
import concourse.tile as tile
from concourse import bacc
from concourse.bass_utils import run_bass_kernel_spmd
from concourse.masks import make_identity

F32 = mybir.dt.float32
F32R = mybir.dt.float32r
EXP = mybir.ActivationFunctionType.Exp
IDENT = mybir.ActivationFunctionType.Identity

B, L, D = 2, 2048, 1024
H, DH = 16, 64          # total heads, head dim
CHUNK = 128
HPC = 4                 # heads per core
S = HPC * DH            # 256 per-core qkv width per projection
N_CORES = 8
LT = 512                # l-tile (i-tile) size
NLT = L // LT           # 4
NKT = D // 128          # 8 k-tiles over D
NCT = 3 * S // 128      # 6 c-tiles (q pair0, q pair1, k p0, k p1, v p0, v p1)
NJT = L // CHUNK        # 16 j-tiles/chunks
SCALE = 1.0 / float(np.sqrt(DH))


def build_program():
    nc = bacc.Bacc("TRN2", target_bir_lowering=False, debug=False)
    x_d = nc.dram_tensor("x", [L, D], F32, kind="ExternalInput")
    w_d = nc.dram_tensor("w_qkv", [D, 3 * S], F32, kind="ExternalInput")
    bq_d = nc.dram_tensor("b_qkv", [3 * S], F32, kind="ExternalInput")
    wo_d = nc.dram_tensor("w_out", [S, D], F32, kind="ExternalInput")
    y_d = nc.dram_tensor("y", [L, D], F32, kind="ExternalOutput")

    with tile.TileContext(nc) as tc:
        lp = nc.allow_low_precision(reason="float32r matmul pipeline")
        lp.__enter__()
        with tc.tile_pool(name="const", bufs=1) as const, \
             tc.tile_pool(name="big", bufs=1) as big, \
             tc.tile_pool(name="stage", bufs=6) as stage, \
             tc.tile_pool(name="xtp", bufs=2) as xtp, \
             tc.tile_pool(name="expp", bufs=6) as expp, \
             tc.tile_pool(name="work", bufs=2) as work, \
             tc.tile_pool(name="small", bufs=2) as small:

            # ---- constants ----
            ident_f = const.tile([128, 128], F32)
            make_identity(nc, ident_f[:])
            identr = const.tile([128, 128], F32R)
            nc.vector.tensor_copy(identr[:], ident_f[:])
            ones_f = const.tile([128, 1], F32)
            nc.vector.memset(ones_f[:], 1.0)
            ones64 = const.tile([1, 64], F32R)
            o64f = const.tile([1, 64], F32)
            nc.vector.memset(o64f[:], 1.0)
            nc.vector.tensor_copy(ones64[:], o64f[:])
            # b_qkv as per-c-tile per-partition bias columns [128, 6]
            bq_sb = const.tile([128, NCT], F32)
            bq_ap = bq_d.ap()
            nc.sync.dma_start(
                out=bq_sb[:],
                in_=bass.AP(tensor=bq_ap.tensor, offset=bq_ap.offset,
                            ap=[[1, 128], [128, NCT]]),
            )

            # ---- persistent activations/weights ----
            w_sb = big.tile([128, NKT, 3 * S], F32R)       # W_qkv k-tiles
            for kt in range(NKT):
                nc.sync.dma_start(out=w_sb[:, kt, :],
                                  in_=w_d[kt * 128:(kt + 1) * 128, :].bitcast(F32R))
            wo_sb = big.tile([128, 2, D], F32R)            # W_out k-tiles (head pairs)
            for p in range(2):
                nc.sync.dma_start(out=wo_sb[:, p, :],
                                  in_=wo_d[p * 128:(p + 1) * 128, :].bitcast(F32R))
            qt_sb = big.tile([128, 2, L], F32R)            # Q^T pair-stacked
            kt_sb = big.tile([128, 2, L], F32R)            # K^T pair-stacked
            v_sb = big.tile([128, HPC, NJT, 65], F32R)     # V + ones col, per head/j-tile
            ot_sb = big.tile([128, 2, L], F32R)            # normalized o^T pair-stacked
            # ones column of V tiles
            nc.vector.tensor_copy(
                v_sb[:, :, :, 64:65],
                bass.AP(tensor=ones_f.tensor, offset=ones_f.offset,
                        ap=ones_f.ap[:1] + [[0, HPC], [0, NJT], [0, 1]]),
            )

            # ================= Phase A: x^T + QKV projection =================
            with tc.tile_pool(name="tp_ps", bufs=3, space="PSUM") as tp_ps, \
                 tc.tile_pool(name="proj_ps", bufs=2, space="PSUM") as proj_ps:
                for lt in range(NLT):
                    l0 = lt * LT
                    xs = []
                    for sp in range(4):
                        xst = stage.tile([128, D], F32R, tag="xs", name=f"xs_{lt}_{sp}")
                        nc.sync.dma_start(
                            out=xst[:],
                            in_=x_d[l0 + sp * 128: l0 + (sp + 1) * 128, :].bitcast(F32R))
                        xs.append(xst)
                    xT = xtp.tile([128, NKT, LT], F32R, tag="xT")
                    for kt in range(NKT):
                        for sp in range(4):
                            tp = tp_ps.tile([128, 128], F32R, tag="tp")
                            nc.tensor.transpose(
                                tp[:], xs[sp][:, kt * 128:(kt + 1) * 128], identr[:])
                            nc.vector.tensor_copy(
                                xT[:, kt, sp * 128:(sp + 1) * 128], tp[:])
                    for ct in range(NCT):
                        pp = proj_ps.tile([128, LT], F32, tag="pp")
                        for kt in range(NKT):
                            nc.tensor.matmul(
                                pp[:], w_sb[:, kt, ct * 128:(ct + 1) * 128],
                                xT[:, kt, :],
                                start=(kt == 0), stop=(kt == NKT - 1))
                        if ct < 2:
                            nc.scalar.activation(qt_sb[:, ct, l0:l0 + LT], pp[:],
                                                 IDENT, bias=bq_sb[:, ct:ct + 1])
                        elif ct < 4:
                            nc.scalar.activation(kt_sb[:, ct - 2, l0:l0 + LT], pp[:],
                                                 IDENT, bias=bq_sb[:, ct:ct + 1])
                        else:
                            pv = ct - 4
                            vt_tmp = work.tile([128, LT], F32R, tag="vt_tmp")
                            nc.scalar.activation(vt_tmp[:], pp[:], IDENT,
                                                 bias=bq_sb[:, ct:ct + 1])
                            for sp in range(4):
                                jt = lt * 4 + sp
                                tp2 = tp_ps.tile([128, 128], F32R, tag="tp")
                                nc.tensor.transpose(
                                    tp2[:], vt_tmp[:, sp * 128:(sp + 1) * 128],
                                    identr[:])
                                nc.vector.tensor_copy(
                                    v_sb[:, 2 * pv, jt, 0:64], tp2[:, 0:64])
                                nc.vector.tensor_copy(
                                    v_sb[:, 2 * pv + 1, jt, 0:64], tp2[:, 64:128])

            # ================= Phase B: attention =================
            with tc.tile_pool(name="s_ps", bufs=4, space="PSUM") as s_ps, \
                 tc.tile_pool(name="o_ps", bufs=2, space="PSUM") as o_psp, \
                 tc.tile_pool(name="rb_ps", bufs=2, space="PSUM") as rb_psp:
                for p in range(2):
                    for t in range(NLT):
                        i0 = t * LT
                        njt = 4 * (t + 1)
                        o_ps = [o_psp.tile([65, LT], F32, tag="o_ps", name=f"o_ps_{p}_{t}_{hh}") for hh in range(2)]
                        for jt in range(njt):
                            s_pair = [s_ps.tile([128, LT], F32, tag="s", name=f"s_{p}_{t}_{jt}_{hh}") for hh in range(2)]
                            for hh in range(2):
                                nc.tensor.matmul(
                                    s_pair[hh][:],
                                    kt_sb[hh * 64:(hh + 1) * 64, p,
                                          jt * 128:(jt + 1) * 128],
                                    qt_sb[hh * 64:(hh + 1) * 64, p, i0:i0 + LT],
                                    start=True, stop=True)
                            vis = max(0, jt - 4 * t) * 128
                            for hh in range(2):
                                h = 2 * p + hh
                                e_t = expp.tile([128, LT], F32R, tag="e_t", name=f"e_{p}_{t}_{jt}_{hh}")
                                if vis > 0:
                                    nc.scalar.activation(
                                        e_t[:, vis:LT], s_pair[hh][:, vis:LT],
                                        EXP, scale=SCALE)
                                    nc.vector.memset(e_t[:, 0:vis].bitcast(F32), 0.0)
                                else:
                                    nc.scalar.activation(e_t[:], s_pair[hh][:],
                                                         EXP, scale=SCALE)
                                nc.tensor.matmul(
                                    o_ps[hh][:], v_sb[:, h, jt, :], e_t[:],
                                    start=(jt == 0), stop=(jt == njt - 1))
                        # normalize: r = 1/colsum, broadcast via K=1 matmul
                        r2 = small.tile([1, 2, LT], F32R, tag="r2")
                        for hh in range(2):
                            nc.vector.reciprocal(r2[:, hh, :], o_ps[hh][64:65, :])
                        for hh in range(2):
                            rb = rb_psp.tile([64, LT], F32, tag="rb")
                            nc.tensor.matmul(rb[:], ones64[:], r2[:, hh, :],
                                             start=True, stop=True)
                            rb_sb = work.tile([64, LT], F32, tag="rb_sb")
                            nc.scalar.copy(rb_sb[:], rb[:])
                            if hh == 0:
                                nc.vector.tensor_mul(
                                    ot_sb[0:64, p, i0:i0 + LT],
                                    o_ps[hh][0:64, :], rb_sb[:])
                            else:
                                oB = work.tile([64, LT], F32R, tag="oB")
                                nc.vector.tensor_mul(oB[:], o_ps[hh][0:64, :],
                                                     rb_sb[:])
                                nc.sync.dma_start(
                                    out=ot_sb[64:128, p, i0:i0 + LT], in_=oB[:])

            # ================= Phase C: out projection (partial) =================
            with tc.tile_pool(name="y_ps", bufs=2, space="PSUM") as y_psp:
                for it in range(L // 128):
                    for mt in range(2):
                        yp = y_psp.tile([128, 512], F32, tag="yp")
                        for p in range(2):
                            nc.tensor.matmul(
                                yp[:], ot_sb[:, p, it * 128:(it + 1) * 128],
                                wo_sb[:, p, mt * 512:(mt + 1) * 512],
                                start=(p == 0), stop=(p == 1))
                        y_sb = work.tile([128, 512], F32, tag="y_sb", name=f"ysb_{it}_{mt}")
                        nc.scalar.copy(y_sb[:], yp[:])
                        nc.sync.dma_start(
                            out=y_d[it * 128:(it + 1) * 128,
                                    mt * 512:(mt + 1) * 512],
                            in_=y_sb[:])
        lp.__exit__(None, None, None)
    nc.compile()
    return nc


_NC_CACHE = {}


def _get_nc():
    if "nc" not in _NC_CACHE:
        _NC_CACHE["nc"] = build_program()
    return _NC_CACHE["nc"]


def make_in_maps(x, W_qkv, b_qkv, W_out):
    x = np.ascontiguousarray(np.asarray(x, dtype=np.float32))
    W_qkv = np.asarray(W_qkv, dtype=np.float32)
    b_qkv = np.asarray(b_qkv, dtype=np.float32)
    W_out = np.asarray(W_out, dtype=np.float32)
    in_maps = []
    for c in range(N_CORES):
        b, g = divmod(c, 4)
        cols = []
        for blk in range(3):          # q, k, v column blocks
            c0 = blk * D + g * S
            cols.append(np.arange(c0, c0 + S))
        cols = np.concatenate(cols)
        in_maps.append({
            "x": np.ascontiguousarray(x[b]),
            "w_qkv": np.ascontiguousarray(W_qkv[:, cols]),
            "b_qkv": np.ascontiguousarray(b_qkv[cols]),
            "w_out": np.ascontiguousarray(W_out[g * S:(g + 1) * S, :]),
        })
    return in_maps


def kernel(x, W_qkv, b_qkv, W_out, b_out):
    nc = _get_nc()
    in_maps = make_in_maps(x, W_qkv, b_qkv, W_out)
    res = run_bass_kernel_spmd(nc, in_maps, list(range(N_CORES)))
    b_out = np.asarray(b_out, dtype=np.float32)
    out = np.zeros((B, L, D), dtype=np.float32)
    for c in range(N_CORES):
        out[c // 4] += res.results[c]["y"]
    out += b_out[None, None, :]
    return out
